# revision 3
# baseline (speedup 1.0000x reference)
"""Trainium2 Bass kernel for a GCN layer (gnn_message_passing).

Reference computation (per batch b):
    node_msg = h @ Wn_w.T + Wn_b                  # (N, OD)
    edge_msg = edge_feat @ We_w.T + We_b          # (N, N, OD)
    self_msg = h @ Ws_w.T + Ws_b                  # (N, OD)
    deg      = adj.sum(-1)                        # (N,)
    agg      = node_msg * deg + einsum('ij,ijo->io', adj, edge_msg)
    out      = relu(agg / clip(deg, 1) + self_msg)

Key algebraic rewrite: the (N,N,OD) edge_msg is never materialized.
    einsum('ij,ijo->io', adj, edge_feat @ We.T + We_b)
      = (einsum('ij,ije->ie', adj, edge_feat)) @ We.T + deg * We_b
so the dominant work is the adj-masked reduction of edge_feat over the
source-node axis j, producing (N, ED), followed by a tiny 16->64 matmul.

Sharding: data-parallel over batch B=8 across the 8 NeuronCores (one
batch element per core); weights replicated.

On-chip strategy per core:
  - edge_feat[b] is streamed in i-blocks of 128 destination nodes as
    (128p, 512j, 16e) tiles: 32 KB contiguous per partition (ideal DMA).
  - masked reduce uses the fused DVE op tensor_tensor_reduce:
        accum_out[i, e] = sum_j ef[i, j, e] * adj_f[i, j]
    one instruction per e-channel (16 per block) -- a single DVE pass
    over the data instead of separate multiply + reduce passes.
  - the (128, 16) masked sums are PE-transposed and projected with We^T
    on the TensorEngine; node/self messages are matmuls with the bias
    folded in via an appended ones-row on h^T.
"""

import os
import sys
from contextlib import ExitStack

import numpy as np


def _ensure_concourse():
    try:
        import concourse  # noqa: F401
        return
    except ImportError:
        pass
    for p in ("/opt/trn_rl_repo", "/root/.axon_site/_ro/trn_rl_repo"):
        if os.path.isdir(p) and p not in sys.path:
            sys.path.insert(0, p)
            try:
                import concourse  # noqa: F401
                return
            except ImportError:
                continue
    raise ImportError("cannot locate the concourse (bass) package")


_ensure_concourse()

import concourse.bacc as bacc  # noqa: E402
import concourse.bass as bass  # noqa: E402
import concourse.tile as tile  # noqa: E402
from concourse import mybir  # noqa: E402
from concourse.bass_utils import run_bass_kernel_spmd  # noqa: E402
from concourse.masks import make_identity  # noqa: E402

B, N, ND, ED, OD = 8, 512, 64, 16, 64
NCORES = 8
PB = 128           # destination-node block (SBUF partitions)
NBLK = N // PB     # 4

F32 = mybir.dt.float32
I32 = mybir.dt.int32


def _row_ap(handle, n):
    """View a 1-D DRAM tensor (n,) as a (1, n) AP."""
    ap = handle.ap()
    return bass.AP(tensor=ap.tensor, offset=ap.offset, ap=[[0, 1], [1, n]])


def build_bass():
    """Build the single-core Bass program (SPMD across 8 cores)."""
    nc = bacc.Bacc(
        "TRN2",
        target_bir_lowering=False,
        debug=False,
        num_devices=NCORES,
    )

    h_d = nc.dram_tensor("h", [N, ND], F32, kind="ExternalInput")
    adj_d = nc.dram_tensor("adj", [N, N], I32, kind="ExternalInput")
    ef_d = nc.dram_tensor("edge_feat", [N, N, ED], F32, kind="ExternalInput")
    wn_d = nc.dram_tensor("Wn_w", [OD, ND], F32, kind="ExternalInput")
    wnb_d = nc.dram_tensor("Wn_b", [OD], F32, kind="ExternalInput")
    we_d = nc.dram_tensor("We_w", [OD, ED], F32, kind="ExternalInput")
    web_d = nc.dram_tensor("We_b", [OD], F32, kind="ExternalInput")
    ws_d = nc.dram_tensor("Ws_w", [OD, ND], F32, kind="ExternalInput")
    wsb_d = nc.dram_tensor("Ws_b", [OD], F32, kind="ExternalInput")
    out_d = nc.dram_tensor("out", [N, OD], F32, kind="ExternalOutput")

    h_ap = h_d.ap()
    adj_ap = adj_d.ap()
    ef_ap = ef_d.ap()
    out_ap = out_d.ap()

    with tile.TileContext(nc) as tc, ExitStack() as ctx:
        consts = ctx.enter_context(tc.tile_pool(name="consts", bufs=1))
        efp = ctx.enter_context(tc.tile_pool(name="efp", bufs=2))
        adjp = ctx.enter_context(tc.tile_pool(name="adjp", bufs=2))
        work = ctx.enter_context(tc.tile_pool(name="work", bufs=2))
        outp = ctx.enter_context(tc.tile_pool(name="outp", bufs=2))
        pset = ctx.enter_context(tc.tile_pool(name="pset", bufs=1, space="PSUM"))
        pmm = ctx.enter_context(tc.tile_pool(name="pmm", bufs=1, space="PSUM"))
        pms = ctx.enter_context(tc.tile_pool(name="pms", bufs=2, space="PSUM"))

        ident = consts.tile([128, 128], F32)
        make_identity(nc, ident)

        # --- weights: transpose on PE; biases folded as extra matmul row ---
        # rhs_n = [Wn_w^T ; Wn_b + We_b]  (65, 64)
        # rhs_s = [Ws_w^T ; Ws_b]         (65, 64)
        # weT   = We_w^T                  (16, 64)
        wn_sb = consts.tile([OD, ND], F32, tag="wload")
        nc.sync.dma_start(out=wn_sb, in_=wn_d.ap())
        ws_sb = consts.tile([OD, ND], F32, tag="wload2")
        nc.sync.dma_start(out=ws_sb, in_=ws_d.ap())
        we_sb = consts.tile([OD, ED], F32, tag="wload3")
        nc.sync.dma_start(out=we_sb, in_=we_d.ap())

        rhs_n = consts.tile([ND + 1, OD], F32)
        rhs_s = consts.tile([ND + 1, OD], F32)
        weT = consts.tile([ED, OD], F32)

        pw = pset.tile([ND, OD], F32, tag="pw")
        nc.tensor.transpose(pw, wn_sb, ident[:ND, :OD])
        nc.scalar.copy(out=rhs_n[0:ND, :], in_=pw)
        pw2 = pset.tile([ND, OD], F32, tag="pw")
        nc.tensor.transpose(pw2, ws_sb, ident[:ND, :OD])
        nc.scalar.copy(out=rhs_s[0:ND, :], in_=pw2)
        pw3 = pset.tile([ED, OD], F32, tag="pw")
        nc.tensor.transpose(pw3, we_sb, ident[:ND, :OD])
        nc.scalar.copy(out=weT, in_=pw3)

        bias_n = consts.tile([1, OD], F32)
        nc.sync.dma_start(out=bias_n, in_=_row_ap(wnb_d, OD))
        bias_e = consts.tile([1, OD], F32)
        nc.sync.dma_start(out=bias_e, in_=_row_ap(web_d, OD))
        nc.vector.tensor_add(rhs_n[ND : ND + 1, :], bias_n, bias_e)
        nc.sync.dma_start(out=rhs_s[ND : ND + 1, :], in_=_row_ap(wsb_d, OD))

        # --- h^T with an appended ones-row: (65, 512) ---
        hT = consts.tile([ND + 1, N], F32)
        nc.vector.memset(hT[ND : ND + 1, :], 1.0)
        for ib in range(NBLK):
            h_sb = work.tile([PB, ND], F32, tag="hload")
            nc.sync.dma_start(out=h_sb, in_=h_ap[ib * PB : (ib + 1) * PB, :])
            ph = pset.tile([ND, PB], F32, tag="ph")
            nc.tensor.transpose(ph, h_sb, ident)
            nc.scalar.copy(out=hT[0:ND, ib * PB : (ib + 1) * PB], in_=ph)

        # --- main loop over destination-node blocks ---
        for ib in range(NBLK):
            i0 = ib * PB

            adj_i = adjp.tile([PB, N], I32, tag="adji")
            nc.sync.dma_start(out=adj_i, in_=adj_ap[i0 : i0 + PB, :])
            adj_f = adjp.tile([PB, N], F32, tag="adjf")
            nc.vector.tensor_copy(out=adj_f, in_=adj_i)

            deg = work.tile([PB, 1], F32, tag="deg")
            nc.vector.reduce_sum(deg, adj_f, axis=mybir.AxisListType.X)
            degc = work.tile([PB, 1], F32, tag="degc")
            nc.vector.tensor_scalar_max(degc, deg, 1.0)
            r = work.tile([PB, 1], F32, tag="r")
            nc.vector.reciprocal(r, degc)
            degr = work.tile([PB, 1], F32, tag="degr")
            nc.vector.tensor_mul(degr, deg, r)

            ef_t = efp.tile([PB, N, ED], F32, tag="ef")
            nc.sync.dma_start(out=ef_t, in_=ef_ap[i0 : i0 + PB, :, :])

            # masked sum over source nodes j, one fused op per e-channel:
            #   ms[:, e] = sum_j ef[:, j, e] * adj_f[:, j]
            # (scalar_tensor_tensor with op0=bypass is a fused
            #  elementwise-multiply + free-dim reduction in one DVE pass)
            ms = work.tile([PB, ED], F32, tag="ms")
            scratch = work.tile([PB, N], F32, tag="scratch")
            for e in range(ED):
                nc.vector.scalar_tensor_tensor(
                    out=scratch,
                    in0=ef_t[:, :, e],
                    scalar=1.0,
                    in1=adj_f,
                    op0=mybir.AluOpType.bypass,
                    op1=mybir.AluOpType.mult,
                    accum_out=ms[:, e : e + 1],
                )

            # (128, 16) -> (16, 128) for the We projection
            pm = pms.tile([ED, PB], F32, tag="pm")
            nc.tensor.transpose(pm, ms, ident)
            msT = work.tile([ED, PB], F32, tag="msT")
            nc.scalar.copy(out=msT, in_=pm)

            pe_ = pmm.tile([PB, OD], F32, tag="pe")
            nc.tensor.matmul(pe_, lhsT=msT, rhs=weT, start=True, stop=True)
            pn = pmm.tile([PB, OD], F32, tag="pn")
            nc.tensor.matmul(
                pn, lhsT=hT[:, i0 : i0 + PB], rhs=rhs_n, start=True, stop=True
            )
            ps = pmm.tile([PB, OD], F32, tag="ps")
            nc.tensor.matmul(
                ps, lhsT=hT[:, i0 : i0 + PB], rhs=rhs_s, start=True, stop=True
            )

            # out = relu(degr * node + r * edge + self)
            acc = outp.tile([PB, OD], F32, tag="acc")
            nc.vector.tensor_scalar_mul(acc, pn, degr)
            acc2 = outp.tile([PB, OD], F32, tag="acc2")
            nc.vector.scalar_tensor_tensor(
                out=acc2,
                in0=pe_,
                scalar=r,
                in1=acc,
                op0=mybir.AluOpType.mult,
                op1=mybir.AluOpType.add,
            )
            ob = outp.tile([PB, OD], F32, tag="ob")
            nc.vector.tensor_add(ob, acc2, ps)
            nc.vector.tensor_relu(out=ob, in_=ob)
            nc.sync.dma_start(out=out_ap[i0 : i0 + PB, :], in_=ob)

    nc.compile()
    return nc


_NC_CACHE = None


def _get_nc():
    global _NC_CACHE
    if _NC_CACHE is None:
        _NC_CACHE = build_bass()
    return _NC_CACHE


def make_in_maps(inputs):
    w = {
        k: np.ascontiguousarray(np.asarray(inputs[k], dtype=np.float32))
        for k in ("Wn_w", "Wn_b", "We_w", "We_b", "Ws_w", "Ws_b")
    }
    h = np.asarray(inputs["h"], dtype=np.float32)
    adj = np.asarray(inputs["adj"], dtype=np.int32)
    ef = np.asarray(inputs["edge_feat"], dtype=np.float32)
    in_maps = []
    for c in range(NCORES):
        m = dict(w)
        m["h"] = np.ascontiguousarray(h[c])
        m["adj"] = np.ascontiguousarray(adj[c])
        m["edge_feat"] = np.ascontiguousarray(ef[c])
        in_maps.append(m)
    return in_maps


def run(inputs, trace=False):
    """Run on hardware; returns (full_output, BassKernelResults)."""
    nc = _get_nc()
    res = run_bass_kernel_spmd(nc, make_in_maps(inputs), list(range(NCORES)), trace=trace)
    out = np.stack(
        [np.asarray(res.results[c]["out"]) for c in range(NCORES)], axis=0
    ).astype(np.float32)
    return out, res


def kernel(**inputs):
    out, _ = run(inputs)
    return out
